# Initial kernel scaffold
#
"""Multi-head self-attention with LoRA projections on 8 Trainium2 NeuronCores.

Problem: nn_MultiHeadSelfAttention (B=2, L=2048, D=1024, H=16, hd=64, LoRA r=16).

Sharding: tensor-parallel on heads for QKV + attention (2 heads/core), then an
AllToAll reshards by token for the output projection (512 tokens/core), so the
final output is a clean concat (no host-side reduction).

Per-core pipeline (bf16 on the PE, fp32 accumulation in PSUM):
  1. Weight prep: W_eff = W^T + 0.5*A@B folded on-chip (W^T via PE matmul
     against identity, LoRA via K=16 matmul), cast to bf16.
     attn scale 1/sqrt(hd) folded into W_eff_q and bq.
  2. x [4096,1024] cast fp32->bf16 via SWDGE DMA (HBM->HBM), then HWDGE
     DMA-transpose into xT [8 x 128 x 4096] in SBUF.
  3. qT/kT = W_eff^T x^T in [out, token] layout (bias added during the
     PSUM->SBUF copy on ScalarE); v in [token, d] layout with a ones column
     appended (softmax row-sums); bv deferred past softmax (rows of softmax
     sum to 1, so P@(v + 1 bv^T) = P@v + bv).
  4. Per (batch, head): S^T tiles [m=128, l=512] in PSUM; attn bias is
     cast-DMA'd to bf16 natural [l, m] and injected into PSUM by PE matmuls
     against identity (= transpose), scores accumulate on top; exp on ScalarE
     -> E^T bf16; AV matmul with lhsT=[v|1] accumulates O'^T [65, l] where
     row 64 is the softmax denominator; finalize: transpose O'^T tiles,
     normalize (DVE reciprocal + per-partition scale), transpose back to
     OT [head_dim, token] adding bv.
  5. AllToAll (one per batch) reshards OT from head-split to token-split.
  6. Output projection y = OT_full^T @ Wo_eff + bo for this core's 512 tokens.

Host side only shards/gathers: slices weights/bias per core, concatenates the
per-core [512, 1024] outputs.
"""

import numpy as np

B = 2
L = 2048
D = 1024
H = 16
HD = 64
R = 16
SCALING = 0.5  # LoRA alpha/r
SCALE = HD ** (-0.5)  # attention scale, folded into Wq_eff/bq

N_CORES = 8
HPC = H // N_CORES  # heads per core = 2
OPC = HPC * HD  # out-dims per core for q/k/v = 128
TOK = B * L  # 4096
TPC = TOK // N_CORES  # tokens per core after AllToAll = 512
TPB = L // N_CORES  # tokens per (core, batch) = 256

_CACHE = {}


def _build_kernel(no_collective=False, num_devices=N_CORES, repeat=1):
    import concourse.tile as tile
    import concourse.mybir as mybir
    from concourse import bacc
    from concourse.masks import make_identity
    from contextlib import ExitStack

    f32 = mybir.dt.float32
    bf16 = mybir.dt.bfloat16
    AF = mybir.ActivationFunctionType

    nc = bacc.Bacc("TRN2", target_bir_lowering=False, debug=False,
                   enable_asserts=False, num_devices=num_devices)

    # ---- per-core external inputs ----
    x_ap = nc.dram_tensor("x", [TOK, D], f32, kind="ExternalInput").ap()
    bias_ap = nc.dram_tensor("bias", [HPC, L, L], f32, kind="ExternalInput").ap()
    w_aps, b_aps, a_aps, lb_aps = {}, {}, {}, {}
    for p in "qkv":
        w_aps[p] = nc.dram_tensor(f"W{p}", [OPC, D], f32, kind="ExternalInput").ap()
        b_aps[p] = nc.dram_tensor(f"b{p}", [OPC, 1], f32, kind="ExternalInput").ap()
        a_aps[p] = nc.dram_tensor(f"A{p}", [D, R], f32, kind="ExternalInput").ap()
        lb_aps[p] = nc.dram_tensor(f"B{p}", [R, OPC], f32, kind="ExternalInput").ap()
    wo_ap = nc.dram_tensor("Wo", [D, D], f32, kind="ExternalInput").ap()
    bo_ap = nc.dram_tensor("bo", [1, D], f32, kind="ExternalInput").ap()
    ao_ap = nc.dram_tensor("Ao", [D, R], f32, kind="ExternalInput").ap()
    lbo_ap = nc.dram_tensor("Bo", [R, D], f32, kind="ExternalInput").ap()

    y_ap = nc.dram_tensor("y", [TPC, D], f32, kind="ExternalOutput").ap()

    KT = D // 128  # 8 contraction k-tiles for the projections
    LT = L // 128  # 16 l-tiles per batch
    MT = L // 128  # 16 m-tiles per batch
    LCH = 1024  # l-chunk (2 PSUM banks; exp runs as one wide ACT op)
    NLC = L // LCH  # 2 l-chunks per batch
    MC = 512  # m-chunk for staged bias
    NMC = L // MC  # 4 m-chunks

    with tile.TileContext(nc) as tc, ExitStack() as top:
        const_pool = top.enter_context(tc.tile_pool(name="const", bufs=1))
        ident = const_pool.tile([128, 128], bf16)
        make_identity(nc, ident[:])
        identf = const_pool.tile([128, 128], f32)
        make_identity(nc, identf[:])
        ones_row = const_pool.tile([1, 128], bf16)
        nc.gpsimd.memset(ones_row[:], 1.0)

        dram = top.enter_context(tc.tile_pool(name="dram", bufs=1, space="DRAM"))

        for rep in range(repeat):
          with ExitStack() as rctx:
            # ================= weight prep =================
            weff_pool = rctx.enter_context(tc.tile_pool(name="weff", bufs=1))
            weff = {p: weff_pool.tile([128, D], bf16, name=f"weff_{p}") for p in "qkv"}
            woeff = weff_pool.tile([128, KT, D], bf16)  # [hd in tile, ktile, out]
            bias_q = const_pool.tile([128, 1], f32)
            bias_k = const_pool.tile([128, 1], f32)
            bv_cat = const_pool.tile([128, 1], f32)
            bo_row = const_pool.tile([1, D], bf16)

            early_bias = rctx.enter_context(tc.tile_pool(name="early_bias", bufs=4))
            qkv_pool = rctx.enter_context(tc.tile_pool(name="qkv", bufs=1))
            qT = qkv_pool.tile([128, TOK], bf16)
            kT = qkv_pool.tile([128, TOK], bf16)
            vsb = qkv_pool.tile([128, TOK // 128, 130], bf16)

            with ExitStack() as xctx:
                xp = xctx.enter_context(tc.tile_pool(name="xT", bufs=1))
                xT = xp.tile([128, KT, TOK], bf16)
                with ExitStack() as wctx:
                    wsm = wctx.enter_context(tc.tile_pool(name="wsm", bufs=4))
                    won_pool = wctx.enter_context(tc.tile_pool(name="won", bufs=1))
                    wps = wctx.enter_context(
                        tc.tile_pool(name="wps", bufs=1, space="PSUM"))

                    # ---- all weight DMAs up front (SWDGE queue fills early) ----
                    wnat, anat, lb = {}, {}, {}
                    for p in "qkv":
                        wnat[p] = wsm.tile([128, D], bf16, tag="wnat", name=f"wn{p}")
                        nc.gpsimd.dma_start(wnat[p][:], w_aps[p][:, :])
                        anat[p] = wsm.tile([128, KT, R], bf16, tag="anat",
                                           name=f"an{p}")
                        nc.gpsimd.dma_start(
                            anat[p][:],
                            a_aps[p].rearrange("(kt p2) r -> p2 kt r", p2=128))
                        lb[p] = wsm.tile([R, 128], bf16, tag="lb", name=f"lb{p}")
                        nc.gpsimd.dma_start(lb[p][:], lb_aps[p][:, :])
                    anat["o"] = wsm.tile([128, KT, R], bf16, tag="anat", name="ano")
                    nc.gpsimd.dma_start(
                        anat["o"][:], ao_ap.rearrange("(kt p2) r -> p2 kt r", p2=128))
                    lbo = wsm.tile([R, D], bf16, tag="lbo")
                    nc.gpsimd.dma_start(lbo[:], lbo_ap[:, :])
                    wonat = won_pool.tile([128, KT, D], bf16)
                    nc.gpsimd.dma_start(
                        wonat[:], wo_ap.rearrange("(oj p2) d -> p2 oj d", p2=128))
                    nc.gpsimd.dma_start(bo_row[:], bo_ap[:, :])  # cast to bf16
                    bq_raw = wsm.tile([128, 1], f32, tag="braw")
                    nc.sync.dma_start(bq_raw[:], b_aps["q"][:, :])
                    nc.sync.dma_start(bias_k[:], b_aps["k"][:, :])
                    nc.sync.dma_start(bv_cat[:], b_aps["v"][:, :])
                    nc.gpsimd.memset(vsb[:, :, 64:65], 1.0)
                    nc.gpsimd.memset(vsb[:, :, 129:130], 1.0)
                    nc.gpsimd.memset(ones_row[:], 1.0)
                    # prefetch h0's first bias chunks during the x/qkv phase
                    MH = 256
                    early_bstage = []
                    for mh in range(4):
                        bt = early_bias.tile([128, LT, MH], bf16, tag="eb",
                                             name=f"eb{mh}")
                        nc.gpsimd.dma_start(
                            bt[:],
                            bias_ap[0].rearrange("(lt p) m -> p lt m", p=128)[
                                :, :, mh * MH:(mh + 1) * MH])
                        early_bstage.append(bt)

                    # ---- weight-prep compute pieces (interleaved into x loop) ----
                    ats = {}

                    def piece_at(p):
                        def run():
                            pat = wps.tile([R, D], bf16, tag="wps", name=f"pat{p}")
                            for kt2 in range(KT):
                                nc.tensor.matmul(pat[:, kt2 * 128:(kt2 + 1) * 128],
                                                 anat[p][:, kt2, :], ident[:],
                                                 is_transpose=True)
                            ats[p] = wsm.tile([R, D], bf16, tag="ats", name=f"at{p}")
                            nc.vector.tensor_scalar_mul(ats[p][:], pat[:], SCALING)
                        return run

                    def piece_proj(p):
                        def run():
                            ps = wps.tile([128, D], f32, tag="wps", name=f"wq{p}")
                            for ki in range(KT):
                                ksl = slice(ki * 128, (ki + 1) * 128)
                                nc.tensor.matmul(ps[:, ksl], wnat[p][:, ksl],
                                                 ident[:], start=True, stop=False,
                                                 skip_group_check=True)
                                nc.tensor.matmul(ps[:, ksl], ats[p][:, ksl],
                                                 lb[p][:], start=False, stop=True,
                                                 skip_group_check=True)
                            sc = SCALE if p == "q" else 1.0
                            nc.vector.tensor_scalar_mul(weff[p][:], ps[:], sc)
                        return run

                    def piece_wo(ki):
                        def run():
                            ksl = slice(ki * 128, (ki + 1) * 128)
                            ps = wps.tile([128, D], f32, tag="wps", name=f"wo{ki}")
                            for oj in range(KT):
                                osl2 = slice(oj * 128, (oj + 1) * 128)
                                nc.tensor.matmul(ps[:, osl2], wonat[:, oj, ksl],
                                                 ident[:], start=True, stop=False,
                                                 skip_group_check=True)
                                nc.tensor.matmul(ps[:, osl2], ats["o"][:, ksl],
                                                 lbo[:, osl2], start=False,
                                                 stop=True, skip_group_check=True)
                            nc.vector.tensor_copy(woeff[:, ki, :], ps[:])
                        return run

                    def piece_bias():
                        def run():
                            nc.vector.tensor_scalar_mul(bias_q[:], bq_raw[:], SCALE)
                        return run

                    # ---- interleaved x-transpose / weight-prep / q,k,v projections ----
                    pp = wctx.enter_context(tc.tile_pool(name="proj_ps", bufs=3,
                                                         space="PSUM"))

                    def qk_chunk(lc):
                        tsl = slice(lc * 512, (lc + 1) * 512)
                        for dst, p, bias_t in ((qT, "q", bias_q), (kT, "k", bias_k)):
                            ps = pp.tile([128, 512], f32, tag="proj",
                                         name=f"pj{p}{lc}")
                            for ki in range(KT):
                                ksl = slice(ki * 128, (ki + 1) * 128)
                                nc.tensor.matmul(ps[:], weff[p][:, ksl],
                                                 xT[:, ki, tsl],
                                                 start=(ki == 0), stop=(ki == KT - 1))
                            nc.vector.tensor_scalar_add(dst[:, tsl], ps[:], bias_t[:])

                    def v_tile(tt):
                        tsl = slice(tt * 128, (tt + 1) * 128)
                        ps = pp.tile([128, 128], f32, tag="proj", name=f"pv{tt}")
                        for ki in range(KT):
                            ksl = slice(ki * 128, (ki + 1) * 128)
                            nc.tensor.matmul(ps[:], xT[:, ki, tsl], weff["v"][:, ksl],
                                             start=(ki == 0), stop=(ki == KT - 1))
                        nc.vector.tensor_copy(vsb[:, tt, 0:64], ps[:, 0:64])
                        nc.vector.tensor_copy(vsb[:, tt, 65:129], ps[:, 64:128])

                    with ExitStack() as xtctx:
                        xload = xtctx.enter_context(
                            tc.tile_pool(name="xload", bufs=3))
                        xps = xtctx.enter_context(
                            tc.tile_pool(name="xps", bufs=2, space="PSUM"))
                        XCH = 2  # token-tiles per cast-DMA chunk
                        NTT = TOK // 128

                        # pieces needed early (weff q/k/v); o/wo/bias deferred
                        early_pieces = {4: piece_at("q"), 5: piece_proj("q"),
                                        6: piece_at("k"), 7: piece_proj("k"),
                                        8: piece_at("v"), 9: piece_proj("v"),
                                        10: piece_bias()}
                        late_pieces = ([piece_at("o")]
                                       + [piece_wo(ki) for ki in range(KT)])
                        qk_pending = list(range(TOK // 512))  # 8 chunks
                        v_pending = list(range(NTT))

                        def fill(tt):
                            # one projection per x-tile slot, once inputs exist
                            if tt < 11:
                                return
                            if qk_pending and qk_pending[0] * 4 + 3 <= tt:
                                qk_chunk(qk_pending.pop(0))
                            elif v_pending and v_pending[0] <= tt:
                                v_tile(v_pending.pop(0))

                        for tt in range(NTT):
                            tc_, to = divmod(tt, XCH)
                            if to == 0:
                                xn = xload.tile([128, XCH, D], bf16, tag="xn")
                                nc.gpsimd.dma_start(
                                    xn[:],
                                    x_ap[tc_ * XCH * 128:(tc_ + 1) * XCH * 128, :]
                                    .rearrange("(c p2) d -> p2 c d", p2=128))
                            xq = xps.tile([128, D], bf16, tag="xq")
                            for ki in range(KT):
                                ksl = slice(ki * 128, (ki + 1) * 128)
                                nc.tensor.matmul(xq[:, ksl], xn[:, to, ksl], ident[:],
                                                 is_transpose=True)
                            nc.vector.tensor_copy(
                                xT[:, :, tt * 128:(tt + 1) * 128],
                                xq[:].rearrange("p (ki t) -> p ki t", ki=KT))
                            if tt in early_pieces:
                                early_pieces[tt]()
                            else:
                                fill(tt)
                        # drain remaining projections, interleaving the deferred
                        # o-projection weight prep
                        while qk_pending or v_pending or late_pieces:
                            if qk_pending:
                                qk_chunk(qk_pending.pop(0))
                            elif late_pieces:
                                late_pieces.pop(0)()
                            for _ in range(2):
                                if v_pending:
                                    v_tile(v_pending.pop(0))

            # ================= attention =================
            ot_pool = rctx.enter_context(tc.tile_pool(name="ot", bufs=2))
            a2a_pool = rctx.enter_context(tc.tile_pool(name="a2a", bufs=2, space="DRAM"))
            y_pool = rctx.enter_context(tc.tile_pool(name="ysb", bufs=2))
            otf_pool = rctx.enter_context(tc.tile_pool(name="otf", bufs=2))

            with ExitStack() as actx:
                bias_pool = actx.enter_context(tc.tile_pool(name="bias_nat", bufs=8))
                psA = actx.enter_context(tc.tile_pool(name="psA", bufs=3, space="PSUM"))
                psB = actx.enter_context(tc.tile_pool(name="psB", bufs=1, space="PSUM"))
                e_pool = actx.enter_context(tc.tile_pool(name="e", bufs=3))
                fin_pool = actx.enter_context(tc.tile_pool(name="fin", bufs=2))
                ocat_pool = actx.enter_context(tc.tile_pool(name="ocat", bufs=2))

                ocats = [ocat_pool.tile([128, LT, 128], bf16, tag="ocat",
                                        name=f"ocat{bx}") for bx in range(B)]

                def attention(h, lc, b, bstage):
                    # one l-chunk of one batch: S^T/exp/AV over all m, then
                    # normalize into ocat rows lc*8..lc*8+7
                    hsl = slice(h * 64, (h + 1) * 64)
                    lof = b * L + lc * LCH
                    qTh = qT[hsl, lof:lof + LCH]
                    kTh = kT[hsl, b * L:(b + 1) * L]
                    po = psB.tile([65, LCH], f32, tag="po", name=f"po{h}{b}{lc}")
                    for mi in range(MT):
                        mh, mo = divmod(mi * 128, MH)
                        ps = psA.tile([128, LCH], f32, tag="ps")
                        for half in range(2):
                            hof = half * 512
                            nc.tensor.matmul(
                                ps[:, hof:hof + 512],
                                kTh[:, mi * 128:(mi + 1) * 128],
                                qTh[:, hof:hof + 512],
                                start=True, stop=False, skip_group_check=True)
                            for j in range(4):
                                lt = lc * (LCH // 128) + half * 4 + j
                                nc.tensor.matmul(
                                    ps[:, hof + j * 128:hof + (j + 1) * 128],
                                    bstage[mh][:, lt, mo:mo + 128],
                                    ident[:], start=False,
                                    stop=(j == 3), skip_group_check=True)
                        e = e_pool.tile([128, LCH], bf16, tag="e")
                        nc.scalar.activation(e[:], ps[:], AF.Exp)
                        for half in range(2):
                            nc.tensor.matmul(
                                po[:, half * 512:(half + 1) * 512],
                                vsb[:, b * MT + mi, h * 65:h * 65 + 65],
                                e[:, half * 512:(half + 1) * 512],
                                start=(mi == 0), stop=(mi == MT - 1),
                                skip_group_check=True)
                    # copy out of PSUM quickly (frees po), then normalize
                    # (split across DVE+ACT so the single po bank frees sooner)
                    stage = fin_pool.tile([65, LCH], f32, tag="st")
                    nc.vector.tensor_copy(stage[:, 0:512], po[:, 0:512])
                    nc.scalar.copy(stage[:, 512:1024], po[:, 512:1024])
                    for j in range(LCH // 128):
                        lt = lc * (LCH // 128) + j
                        pf = psA.tile([128, 65], f32, tag="ps", name=f"pf{h}{b}{lt}")
                        nc.tensor.matmul(pf[:], stage[:, j * 128:(j + 1) * 128],
                                         identf[0:65, 0:65], is_transpose=True)
                        rec = fin_pool.tile([128, 1], f32, tag="rec")
                        nc.vector.reciprocal(rec[:], pf[:, 64:65])
                        nc.vector.tensor_scalar_mul(
                            ocats[b][:, lt, hsl], pf[:, 0:64], rec[:])

                otfs = {}
                a2a_in = a2a_pool.tile([N_CORES, 128, B, TPB], bf16, tag="ain")
                a2a_out = a2a_pool.tile([N_CORES, 128, B, TPB], bf16, tag="aout")

                def finish_comm(b):
                    # transpose ocat -> OT [hd 128, l] and add bv; stage into
                    # this batch's half of the (single, merged) AllToAll buffer
                    ot = ot_pool.tile([128, L], bf16, tag="ot")
                    for lt in range(LT):
                        pt = psA.tile([128, 128], bf16, tag="ps", name=f"pt{b}{lt}")
                        nc.tensor.matmul(pt[:], ocats[b][:, lt, :], ident[:],
                                         is_transpose=True)
                        nc.vector.tensor_scalar_add(
                            ot[:, lt * 128:(lt + 1) * 128], pt[:], bv_cat[:])
                    nc.sync.dma_start(
                        a2a_in[:, :, b, :].rearrange("j p t -> p j t"), ot[:])

                def finish_a2a():
                    # single AllToAll: head-split -> token-split, both batches
                    if no_collective:
                        nc.sync.dma_start(a2a_out[:], a2a_in[:])
                    else:
                        nc.gpsimd.collective_compute(
                            "AllToAll", mybir.AluOpType.bypass,
                            replica_groups=[list(range(N_CORES))],
                            ins=[a2a_in.opt()], outs=[a2a_out.opt()])
                    otf = otf_pool.tile([128, N_CORES, B, TPB], bf16, tag="otf")
                    nc.sync.dma_start(otf[:],
                                      a2a_out[:].rearrange("j p b t -> p j b t"))
                    otfs[0] = otf

                def finish_oproj(b):
                    otf = otfs[0]
                    # ---- output projection for this batch's 256 tokens ----
                    for tt in range(TPB // 128):
                        tsl = slice(tt * 128, (tt + 1) * 128)
                        for nch in range(2):
                            nsl = slice(nch * 512, (nch + 1) * 512)
                            ps = psA.tile([128, 512], f32, tag="ps",
                                          name=f"psy{b}{tt}{nch}")
                            for ki in range(KT):
                                nc.tensor.matmul(
                                    ps[:], otf[:, ki, b, tsl], woeff[:, ki, nsl],
                                    start=(ki == 0), stop=False, skip_group_check=True)
                            nc.tensor.matmul(ps[:], ones_row[:], bo_row[:, nsl],
                                             start=False, stop=True,
                                             skip_group_check=True)
                            ysb = y_pool.tile([128, 512], f32, tag="y")
                            nc.vector.tensor_copy(ysb[:], ps[:])
                            nc.sync.dma_start(
                                y_ap[b * TPB + tt * 128: b * TPB + (tt + 1) * 128, nsl],
                                ysb[:])

                for h in range(HPC):
                    bstage = list(early_bstage) if h == 0 else []
                    for mh in range(len(bstage), L // MH):
                        bt = bias_pool.tile([128, LT, MH], bf16, tag="bn",
                                            name=f"bn{h}{mh}")
                        nc.gpsimd.dma_start(
                            bt[:],
                            bias_ap[h].rearrange("(lt p) m -> p lt m", p=128)[
                                :, :, mh * MH:(mh + 1) * MH],
                        )
                        bstage.append(bt)
                    last = (h == HPC - 1)
                    for lc in range(NLC):
                        for b in range(B):
                            if last and lc == NLC - 1 and b == 1:
                                # batch 0 fully done: stage its A2A half early
                                finish_comm(0)
                            attention(h, lc, b, bstage)
                finish_comm(1)
                finish_a2a()
                finish_oproj(0)
                finish_oproj(1)

    nc.compile()
    return nc


def _shard_inputs(inputs):
    x = np.ascontiguousarray(inputs["x"].reshape(TOK, D))
    attn_bias = inputs["attn_bias"]
    in_maps = []
    for c in range(N_CORES):
        hsl = slice(c * HPC, (c + 1) * HPC)
        osl = slice(c * OPC, (c + 1) * OPC)
        m = {
            "x": x,
            "bias": attn_bias[0, hsl],
            "Wo": inputs["Wo"],
            "bo": inputs["bo"][None, :],
            "Ao": inputs["Ao"],
            "Bo": inputs["Bo"],
        }
        for p in "qkv":
            m[f"W{p}"] = inputs[f"W{p}"][osl]
            m[f"b{p}"] = inputs[f"b{p}"][osl][:, None]
            m[f"A{p}"] = inputs[f"A{p}"]
            m[f"B{p}"] = inputs[f"B{p}"][:, osl]
        in_maps.append(m)
    return in_maps


def _gather_outputs(results):
    y = np.empty((B, L, D), np.float32)
    for c in range(N_CORES):
        yc = results[c]["y"]
        for b in range(B):
            y[b, c * TPB:(c + 1) * TPB] = yc[b * TPB:(b + 1) * TPB]
    return y


def get_nc(**kw):
    key = ("nc", tuple(sorted(kw.items())))
    if key not in _CACHE:
        _CACHE[key] = _build_kernel(**kw)
    return _CACHE[key]


def _get_runner():
    """Cached jitted SPMD executable (avoids re-tracing on repeated calls)."""
    if "runner" in _CACHE:
        return _CACHE["runner"]
    import jax
    from jax.sharding import Mesh, PartitionSpec
    from jax.experimental.shard_map import shard_map
    import concourse.mybir as mybir
    from concourse.bass2jax import (_bass_exec_p, install_neuronx_cc_hook,
                                    partition_id_tensor)

    nc = get_nc()
    install_neuronx_cc_hook()
    partition_name = nc.partition_id_tensor.name if nc.partition_id_tensor else None
    in_names, out_names, out_avals, zero_outs = [], [], [], []
    for alloc in nc.m.functions[0].allocations:
        if not isinstance(alloc, mybir.MemoryLocationSet):
            continue
        name = alloc.memorylocations[0].name
        if alloc.kind == "ExternalInput":
            if name != partition_name:
                in_names.append(name)
        elif alloc.kind == "ExternalOutput":
            shape = tuple(alloc.tensor_shape)
            dtype = mybir.dt.np(alloc.dtype)
            out_names.append(name)
            out_avals.append(jax.core.ShapedArray(shape, dtype))
            zero_outs.append(np.zeros(shape, dtype))
    n_params = len(in_names)
    n_outs = len(out_avals)
    all_in_names = list(in_names) + list(out_names)
    if partition_name is not None:
        all_in_names.append(partition_name)

    def _body(*args):
        operands = list(args)
        if partition_name is not None:
            operands.append(partition_id_tensor())
        outs = _bass_exec_p.bind(
            *operands,
            out_avals=tuple(out_avals),
            in_names=tuple(all_in_names),
            out_names=tuple(out_names),
            lowering_input_output_aliases=(),
            sim_require_finite=True,
            sim_require_nnan=True,
            nc=nc,
        )
        return tuple(outs)

    devices = jax.devices()[:N_CORES]
    mesh = Mesh(np.asarray(devices), ("core",))
    in_specs = (PartitionSpec("core"),) * (n_params + n_outs)
    out_specs = (PartitionSpec("core"),) * n_outs
    fn = jax.jit(shard_map(_body, mesh=mesh, in_specs=in_specs,
                           out_specs=out_specs, check_rep=False),
                 keep_unused=True)
    _CACHE["runner"] = (fn, in_names, out_names, zero_outs)
    return _CACHE["runner"]


def run_on_device(in_maps):
    import jax
    fn, in_names, out_names, zero_outs = _get_runner()
    concat_in = [np.concatenate([np.asarray(in_maps[c][nm])
                                 for c in range(N_CORES)], axis=0)
                 for nm in in_names]
    concat_zeros = [np.zeros((N_CORES * z.shape[0], *z.shape[1:]), z.dtype)
                    for z in zero_outs]
    out = fn(*concat_in, *concat_zeros)
    jax.block_until_ready(out)
    results = []
    for c in range(N_CORES):
        d = {}
        for i, nm in enumerate(out_names):
            arr = np.asarray(out[i])
            per = arr.shape[0] // N_CORES
            d[nm] = arr[c * per:(c + 1) * per]
        results.append(d)
    return results


def kernel(**inputs) -> np.ndarray:
    in_maps = _shard_inputs(inputs)
    results = run_on_device(in_maps)
    return _gather_outputs(results)



# revision 2
# speedup vs baseline: 23.5255x; 23.5255x over previous
"""Multi-head self-attention with LoRA projections on 8 Trainium2 NeuronCores.

Problem: nn_MultiHeadSelfAttention (B=2, L=2048, D=1024, H=16, hd=64, LoRA r=16).

Sharding (ZERO-collective): query-token parallel. Core c owns the 512 query
tokens [qb*512,(qb+1)*512) of batch b, where b = c//4, qb = c%4. Each core
computes K/V for its whole batch locally (replicated across the 4 cores of
that batch group) — this trades ~55us of extra PE time for eliminating the
AllToAll collective (multiple ms in this environment) and makes the final
output a clean per-core concat. Weights/LoRA factors are replicated.

Per-core pipeline (bf16 on the PE, fp32 accumulation in PSUM):
  1. Weff_p = W_p^T + 0.5*A_p@B_p folded on-chip: W^T arrives as a host
     layout-transpose, the rank-16 LoRA product is a PE matmul accumulated
     in PSUM and added in-place on DVE. Attention scale 1/8 at Q eviction.
  2. kT = Weff_k^T x^T in [out, tok] layout (+bk at ACT eviction), streamed
     by 512-token x chunks; qT likewise for the core's own 512 tokens; V in
     [tok, out] layout with a ones column per head (softmax row sums); bv
     deferred past softmax (softmax rows sum to 1).
  3. Attention runs in HEAD PAIRS with two interleaved dependency chains:
     per key tile, S^T [m=128, l=512] in PSUM (contract hd=64); bias
     (host-pre-transposed [m,l] bf16) added on DVE for most tiles and
     PE-injected (identity-matmul accumulate) for 5/16 tiles to balance
     engines; one 1024-wide exp per pair on ACT; AV accumulates O'^T
     [65, 512] in half-chains (row 64 = softmax denominator) so the PSUM
     ring stays small. V's second half and the o-projection weight prep run
     as PE filler between pairs.
  4. Finalize per head: PE-transpose + DVE reciprocal normalize, transpose
     back, +bv; O^T overwrites the dead qT region (no extra SBUF).
  5. y = O^T.T @ Weff_o + bo (bo via rank-1 ones matmul), fp32 out.

Host side only shards/casts/layout-transposes/concats: slices x/bias per
core, casts to bf16, pre-transposes W/A/x/bias (layout only — all arithmetic
including the LoRA fold stays on device), concatenates the per-core
[512, 1024] fp32 outputs into [2, 2048, 1024].
"""

import numpy as np
import ml_dtypes

BF16 = ml_dtypes.bfloat16

B = 2
L = 2048
D = 1024
H = 16
HD = 64
R = 16
SCALING = 0.5  # LoRA alpha/r
SCALE = HD ** (-0.5)  # attention scale, applied at Q eviction

N_CORES = 8
QPC = 512  # query tokens per core
KT = D // 128  # 8 contraction tiles
MT = L // 128  # 16 key tiles per batch
LTQ = QPC // 128  # 4 query tiles per core

_CACHE = {}


def _build_kernel(num_devices=N_CORES, repeat=1):
    import concourse.tile as tile
    import concourse.mybir as mybir
    from concourse import bacc
    from concourse.masks import make_identity
    from contextlib import ExitStack

    f32 = mybir.dt.float32
    bf16 = mybir.dt.bfloat16
    AF = mybir.ActivationFunctionType
    ALU = mybir.AluOpType

    nc = bacc.Bacc("TRN2", target_bir_lowering=False, debug=False,
                   enable_asserts=False, num_devices=num_devices)

    # ---- per-core external inputs (bf16 pre-cast / layout-prepped on host) --
    xbT_ap = nc.dram_tensor("xbT", [D, L], bf16, kind="ExternalInput").ap()
    xqT_ap = nc.dram_tensor("xqT", [D, QPC], bf16, kind="ExternalInput").ap()
    biasT_ap = nc.dram_tensor("biasT", [H, L, QPC], bf16,
                              kind="ExternalInput").ap()
    wt_aps, at_aps, lb_aps = {}, {}, {}
    for p in "qkvo":
        wt_aps[p] = nc.dram_tensor(f"WT{p}", [D, D], bf16,
                                   kind="ExternalInput").ap()
        at_aps[p] = nc.dram_tensor(f"AT{p}", [R, D], bf16,
                                   kind="ExternalInput").ap()
        lb_aps[p] = nc.dram_tensor(f"B{p}", [R, D], bf16,
                                   kind="ExternalInput").ap()
    bq_ap = nc.dram_tensor("bq", [D, 1], f32, kind="ExternalInput").ap()
    bk_ap = nc.dram_tensor("bk", [D, 1], f32, kind="ExternalInput").ap()
    bv2_ap = nc.dram_tensor("bv2", [D, 1], f32, kind="ExternalInput").ap()
    bo_ap = nc.dram_tensor("bo", [1, D], f32, kind="ExternalInput").ap()

    y_ap = nc.dram_tensor("y", [QPC, D], f32, kind="ExternalOutput").ap()

    with tile.TileContext(nc) as tc, ExitStack() as top:
        const_pool = top.enter_context(tc.tile_pool(name="const", bufs=1))
        ident = const_pool.tile([128, 128], bf16)
        make_identity(nc, ident[:])
        identf = const_pool.tile([128, 128], f32)
        make_identity(nc, identf[:])
        ones_row = const_pool.tile([1, 128], bf16)
        nc.gpsimd.memset(ones_row[:], 1.0)

        for rep in range(repeat):
          with ExitStack() as rctx:
            qkv_pool = rctx.enter_context(tc.tile_pool(name="qkv", bufs=1))
            kT = qkv_pool.tile([128, KT, L], bf16)          # K^T [out, tok]
            vsb = qkv_pool.tile([128, MT, H * 65], bf16)    # V [tok, h|1]
            qT = qkv_pool.tile([128, KT, QPC], bf16)  # Q^T; becomes O^T+bv
            bias_vec = const_pool.tile([128, KT, 3], f32, name=f"bvec{rep}")
            # bias_vec[:, kt, 0..2] = bq*SCALE | bk | bv
            bo_row = const_pool.tile([1, D], bf16, name=f"bo{rep}")

            weff_pool = rctx.enter_context(tc.tile_pool(name="weff", bufs=3))
            lsm = rctx.enter_context(tc.tile_pool(name="lsm", bufs=1))
            lora_sm = rctx.enter_context(tc.tile_pool(name="lora", bufs=2))
            xts = rctx.enter_context(tc.tile_pool(name="xts", bufs=2))
            bias_pool = rctx.enter_context(tc.tile_pool(name="bias", bufs=5))
            sadd_pool = rctx.enter_context(tc.tile_pool(name="sadd", bufs=3))
            e_pool = rctx.enter_context(tc.tile_pool(name="e", bufs=4))
            fin_pool = rctx.enter_context(tc.tile_pool(name="fin", bufs=4))
            fin2_pool = rctx.enter_context(tc.tile_pool(name="fin2", bufs=2))
            y_pool = rctx.enter_context(tc.tile_pool(name="ysb", bufs=1))
            sc = rctx.enter_context(tc.tile_pool(name="scps", bufs=4,
                                                 space="PSUM"))
            mm = rctx.enter_context(tc.tile_pool(name="mmps", bufs=2,
                                                 space="PSUM"))
            po_pool = rctx.enter_context(tc.tile_pool(name="pops", bufs=2,
                                                      space="PSUM"))

            # vsb ones columns (disjoint from V evictions; runs immediately)
            ones_cols = vsb[:].rearrange("p m (h e) -> p m h e", e=65)
            nc.gpsimd.memset(ones_cols[:, :, :, 64:65], 1.0)

            ats, lb, weff = {}, {}, {}

            def lora_factors(p, eng=None):
                eng = eng or nc.sync
                lb[p] = lora_sm.tile([R, D], bf16, tag="lb", name=f"lb{p}{rep}")
                eng.dma_start(lb[p][:], lb_aps[p][:, :])
                araw = lora_sm.tile([R, D], bf16, tag="araw", name=f"ar{p}{rep}")
                eng.dma_start(araw[:], at_aps[p][:, :])
                ats[p] = lora_sm.tile([R, D], bf16, tag="ats", name=f"at{p}{rep}")
                nc.gpsimd.tensor_scalar_mul(ats[p][:], araw[:], SCALING)

            for p in "kqv":
                lora_factors(p)

            # ---- small DMAs ----
            braw = lsm.tile([128, KT, 3], f32, name=f"braw{rep}")
            nc.sync.dma_start(
                braw[:, :, 0:1], bq_ap.rearrange("(kt p) o -> p kt o", p=128))
            nc.sync.dma_start(
                braw[:, :, 1:2], bk_ap.rearrange("(kt p) o -> p kt o", p=128))
            nc.sync.dma_start(
                braw[:, :, 2:3], bv2_ap.rearrange("(kt p) o -> p kt o", p=128))
            nc.gpsimd.dma_start(bo_row[:], bo_ap[:, :])  # cast f32->bf16
            nc.vector.tensor_scalar_mul(bias_vec[:, :, 0:1],
                                        braw[:, :, 0:1], SCALE)
            nc.vector.tensor_copy(bias_vec[:, :, 1:3], braw[:, :, 1:3])

            def weff_dma(p):
                weff[p] = weff_pool.tile([128, KT, D], bf16, tag="we",
                                         name=f"we{p}{rep}")
                for ki in range(KT):
                    ksl = slice(ki * 128, (ki + 1) * 128)
                    nc.sync.dma_start(
                        weff[p][:, ki, :],
                        wt_aps[p][ksl, :].rearrange("(o p2) c -> p2 o c",
                                                    p2=128))

            def weff_fold(p):
                for ki in range(KT):
                    ksl = slice(ki * 128, (ki + 1) * 128)
                    for oc in range(2):
                        osl = slice(oc * 512, (oc + 1) * 512)
                        ps = mm.tile([128, 512], f32, tag="mm",
                                     name=f"wf{p}{ki}{oc}")
                        nc.tensor.matmul(ps[:], ats[p][:, ksl], lb[p][:, osl])
                        nc.vector.scalar_tensor_tensor(
                            weff[p][:, ki, osl], ps[:], 1.0,
                            weff[p][:, ki, osl], ALU.mult, ALU.add)

            # DMA queue order tuned so PE never waits long: Wk, x-chunk0,
            # xq, Wq, Wv, then remaining x chunks
            weff_dma("k")
            xc0 = xts.tile([128, KT, 512], bf16, tag="xc", name=f"xc0{rep}")
            nc.sync.dma_start(
                xc0[:], xbT_ap[:, 0:512].rearrange("(ki p2) t -> p2 ki t",
                                                   p2=128))
            xqT = xts.tile([128, KT, QPC], bf16, tag="xc", name=f"xq{rep}")
            nc.sync.dma_start(
                xqT[:], xqT_ap.rearrange("(ki p2) t -> p2 ki t", p2=128))
            weff_dma("q")
            weff_dma("v")

            weff_fold("k")

            def k_chunk(tcc, xTc):
                for ot in range(KT):
                    osl = slice(ot * 128, (ot + 1) * 128)
                    ps = mm.tile([128, 512], f32, tag="mm",
                                 name=f"pk{tcc}{ot}")
                    for ki in range(KT):
                        nc.tensor.matmul(ps[:], weff["k"][:, ki, osl],
                                         xTc[:, ki, :],
                                         start=(ki == 0), stop=(ki == KT - 1),
                                         skip_group_check=True)
                    nc.scalar.add(kT[:, ot, tcc * 512:(tcc + 1) * 512],
                                  ps[:], bias_vec[:, ot, 1:2])

            def v_chunk(tcc, xTc, oc):
                for tw in range(4):
                    tt = tcc * 4 + tw
                    twsl = slice(tw * 128, (tw + 1) * 128)
                    ps = mm.tile([128, 512], f32, tag="mm",
                                 name=f"pv{tt}{oc}")
                    for ki in range(KT):
                        nc.tensor.matmul(
                            ps[:], xTc[:, ki, twsl],
                            weff["v"][:, ki, oc * 512:(oc + 1) * 512],
                            start=(ki == 0), stop=(ki == KT - 1),
                            skip_group_check=True)
                    dst = vsb[:, tt, oc * 520:(oc + 1) * 520].rearrange(
                        "p (h e) -> p h e", e=65)
                    nc.scalar.copy(
                        dst[:, :, 0:64],
                        ps[:].rearrange("p (h e) -> p h e", e=64))

            k_chunk(0, xc0)
            weff_fold("q")
            for ot in range(KT):  # Q^T (scale + bq at ACT eviction)
                osl = slice(ot * 128, (ot + 1) * 128)
                ps = mm.tile([128, QPC], f32, tag="mm", name=f"pq{ot}")
                for ki in range(KT):
                    nc.tensor.matmul(ps[:], weff["q"][:, ki, osl],
                                     xqT[:, ki, :],
                                     start=(ki == 0), stop=(ki == KT - 1),
                                     skip_group_check=True)
                nc.scalar.activation(qT[:, ot, :], ps[:], AF.Identity,
                                     bias=bias_vec[:, ot, 0:1], scale=SCALE)
            weff_fold("v")
            v_chunk(0, xc0, 0)
            for tcc in range(1, 4):
                xTc = xts.tile([128, KT, 512], bf16, tag="xc",
                               name=f"xcA{tcc}")
                nc.sync.dma_start(
                    xTc[:],
                    xbT_ap[:, tcc * 512:(tcc + 1) * 512]
                    .rearrange("(ki p2) t -> p2 ki t", p2=128))
                k_chunk(tcc, xTc)
                v_chunk(tcc, xTc, 0)

            # ---- deferred filler jobs (run interleaved between heads) ----
            vstate = {}

            def v_oc1(tt):
                def run():
                    tcc, tw = divmod(tt, 4)
                    if tw == 0:
                        xc = xts.tile([128, KT, 512], bf16, tag="xc",
                                      name=f"xc1{tt}")
                        nc.gpsimd.dma_start(
                            xc[:],
                            xbT_ap[:, tcc * 512:(tcc + 1) * 512]
                            .rearrange("(ki p2) t -> p2 ki t", p2=128))
                        vstate["xc"] = xc
                    xc = vstate["xc"]
                    twsl = slice(tw * 128, (tw + 1) * 128)
                    ps = mm.tile([128, 512], f32, tag="mm", name=f"pw{tt}")
                    for ki in range(KT):
                        nc.tensor.matmul(ps[:], xc[:, ki, twsl],
                                         weff["v"][:, ki, 512:1024],
                                         start=(ki == 0), stop=(ki == KT - 1),
                                         skip_group_check=True)
                    dst = vsb[:, tt, 520:1040].rearrange("p (h e) -> p h e",
                                                         e=65)
                    nc.scalar.copy(
                        dst[:, :, 0:64],
                        ps[:].rearrange("p (h e) -> p h e", e=64))
                return run

            def wo_load_fold_piece(ki):
                def run():
                    if ki == 0:
                        lora_factors("o", nc.gpsimd)
                        weff["o"] = weff_pool.tile([128, KT, D], bf16,
                                                   tag="we", name=f"weo{rep}")
                    ksl = slice(ki * 128, (ki + 1) * 128)
                    nc.gpsimd.dma_start(
                        weff["o"][:, ki, :],
                        wt_aps["o"][ksl, :].rearrange("(o p2) c -> p2 o c",
                                                      p2=128))
                    for oc in range(2):
                        osl = slice(oc * 512, (oc + 1) * 512)
                        ps = mm.tile([128, 512], f32, tag="mm",
                                     name=f"wo{ki}{oc}")
                        nc.tensor.matmul(ps[:], ats["o"][:, ksl],
                                         lb["o"][:, osl])
                        nc.vector.scalar_tensor_tensor(
                            weff["o"][:, ki, osl], ps[:], 1.0,
                            weff["o"][:, ki, osl], ALU.mult, ALU.add)
                return run

            fillers = [v_oc1(tt) for tt in range(MT)]
            fillers += [wo_load_fold_piece(ki) for ki in range(KT)]

            # ---- attention: head PAIRS interleaved (two chains in flight),
            # AV split into half-accumulators so PSUM po ring stays at 2 ----
            def bias_dma(h, q):
                bt = bias_pool.tile([128, 4, QPC], bf16, tag="bn",
                                    name=f"bn{h}{q}")
                nc.sync.dma_start(
                    bt[:],
                    biasT_ap[h][q * 512:(q + 1) * 512, :]
                    .rearrange("(mt p) l -> p mt l", p=128))
                return bt

            def finalize_pieces(hp, hstages):
                # small closures (ssum per head, then per-j normalize
                # chains), emitted spread across the NEXT pair's iterations
                # so the DVE queue never blocks on the finalize chain
                ssums = {}

                def mk_ssum(g):
                    def run():
                        st0, st1 = hstages[g]
                        ssums[g] = fin2_pool.tile([65, QPC], f32, tag="ss",
                                                  name=f"ss{hp}{g}")
                        nc.vector.scalar_tensor_tensor(
                            ssums[g][:], st0[:], 1.0, st1[:],
                            ALU.mult, ALU.add)
                    return run

                def mk_chain(g, j):
                    def run():
                        h = 2 * hp + g
                        hpo = g * 64
                        pf = sc.tile([128, QPC], f32, tag="ps",
                                     name=f"pf{h}{j}")
                        nc.tensor.matmul(pf[:, 0:65],
                                         ssums[g][:, j * 128:(j + 1) * 128],
                                         identf[0:65, 0:65],
                                         is_transpose=True)
                        rec = fin2_pool.tile([128, 1], f32, tag="rec")
                        nc.vector.reciprocal(rec[:], pf[:, 64:65])
                        otmp = fin2_pool.tile([128, 64], f32, tag="ot")
                        nc.scalar.mul(otmp[:], pf[:, 0:64], rec[:])
                        ptr = sc.tile([128, QPC], f32, tag="ps",
                                      name=f"ptr{h}{j}")
                        nc.tensor.matmul(ptr[0:64, 0:128], otmp[:],
                                         identf[:], is_transpose=True)
                        nc.scalar.add(
                            qT[hpo:hpo + 64, hp, j * 128:(j + 1) * 128],
                            ptr[0:64, 0:128],
                            bias_vec[hpo:hpo + 64, hp, 2:3])
                    return run

                return ([mk_ssum(g) for g in range(2)]
                        + [mk_chain(g, j) for g in range(2)
                           for j in range(LTQ)])

            fi = 0

            def attention_pair(hp, bias_q, prev_fin, filler_budget):
                h0 = 2 * hp
                kTg = [kT[0:64, hp, :], kT[64:128, hp, :]]
                qTg = [qT[0:64, hp, :], qT[64:128, hp, :]]
                povs = {}
                hstages = {0: [], 1: []}
                pend = None
                nonlocal fi

                def emit_av(mt, egs):
                    half = mt // 8
                    for g in range(2):
                        if (g, half) not in povs:
                            povs[(g, half)] = po_pool.tile(
                                [65, QPC], f32, tag="po",
                                name=f"po{h0 + g}h{half}")
                        nc.tensor.matmul(
                            povs[(g, half)][:],
                            vsb[:, mt, (h0 + g) * 65:(h0 + g) * 65 + 65],
                            egs[g], start=(mt % 8 == 0),
                            stop=(mt % 8 == 7), skip_group_check=True)
                    if mt % 8 == 7:  # evict half-accumulators
                        for g in range(2):
                            st = fin_pool.tile([65, QPC], f32, tag="st",
                                               name=f"st{h0 + g}{half}")
                            eng = nc.vector if g == 0 else nc.scalar
                            if g == 0:
                                nc.vector.tensor_copy(st[:],
                                                      povs[(g, half)][:])
                            else:
                                nc.scalar.copy(st[:], povs[(g, half)][:])
                            hstages[g].append(st)

                for mt in range(MT):
                    if prev_fin is not None and mt == 2:
                        for piece in prev_fin:
                            piece()
                    if mt in (4, 7, 10, 13) and fi < len(fillers) \
                            and filler_budget > 0:
                        fillers[fi]()
                        fi += 1
                        filler_budget -= 1
                    if mt in (2, 5, 8, 11, 14):
                        # PE-injected bias: no DVE work for this key tile
                        e2 = e_pool.tile([128, 2, QPC], bf16, tag="e")
                        for g in range(2):
                            ps = sc.tile([128, QPC], f32, tag="ps")
                            bias_t = bias_q[(g, mt // 4)]
                            nc.tensor.matmul(ps[:], ident[:],
                                             bias_t[:, mt % 4, :],
                                             start=True, stop=False,
                                             skip_group_check=True)
                            nc.tensor.matmul(
                                ps[:], kTg[g][:, mt * 128:(mt + 1) * 128],
                                qTg[g][:], start=False, stop=True,
                                skip_group_check=True)
                            nc.scalar.activation(e2[:, g, :], ps[:], AF.Exp)
                        if pend is not None:
                            emit_av(*pend)
                        pend = (mt, [e2[:, 0, :], e2[:, 1, :]])
                        continue
                    sadd = sadd_pool.tile([128, 2, QPC], f32, tag="sa")
                    for g in range(2):
                        ps = sc.tile([128, QPC], f32, tag="ps")
                        nc.tensor.matmul(ps[:],
                                         kTg[g][:, mt * 128:(mt + 1) * 128],
                                         qTg[g][:], start=True, stop=True)
                        bias_t = bias_q[(g, mt // 4)]
                        nc.vector.scalar_tensor_tensor(
                            sadd[:, g, :], ps[:], 1.0,
                            bias_t[:, mt % 4, :], ALU.mult, ALU.add)
                    e2 = e_pool.tile([128, 2, QPC], bf16, tag="e")
                    nc.scalar.activation(e2[:], sadd[:], AF.Exp)
                    if pend is not None:
                        emit_av(*pend)
                    pend = (mt, [e2[:, 0, :], e2[:, 1, :]])
                emit_av(*pend)
                return finalize_pieces(hp, hstages)

            # bias quarter prefetch management: DMA pair hp's quarters just
            # before the pair runs; ring of 6 gives one-pair-ahead prefetch
            bias_store = {}

            def stage_bias(hp):
                m = {}
                for q in range(4):
                    for g in range(2):
                        m[(g, q)] = bias_dma(2 * hp + g, q)
                return m

            bias_store[0] = stage_bias(0)
            pending_fin = None
            for hp in range(KT):
                if hp + 1 < KT:
                    bias_store[hp + 1] = stage_bias(hp + 1)
                budget = 4 if hp < 4 else 2
                pending_fin = attention_pair(hp, bias_store.pop(hp),
                                             pending_fin, budget)
            for piece in pending_fin:
                piece()
            while fi < len(fillers):
                fillers[fi]()
                fi += 1

            # ---- output projection y = O^T.T @ weffo + bo (OT aliased
            # into qT) ----
            for tt in range(LTQ):
                tsl = slice(tt * 128, (tt + 1) * 128)
                pys = [mm.tile([128, 512], f32, tag="mm",
                               name=f"py{tt}{oc}") for oc in range(2)]
                for ki in range(KT):
                    for oc in range(2):
                        nc.tensor.matmul(
                            pys[oc], qT[:, ki, tsl],
                            weff["o"][:, ki, oc * 512:(oc + 1) * 512],
                            start=(ki == 0), stop=False,
                            skip_group_check=True)
                for oc in range(2):
                    osl = slice(oc * 512, (oc + 1) * 512)
                    nc.tensor.matmul(pys[oc], ones_row[:], bo_row[:, osl],
                                     start=False, stop=True,
                                     skip_group_check=True)
                    ysb = y_pool.tile([128, 512], f32, tag="y")
                    nc.vector.tensor_copy(ysb[:], pys[oc])
                    nc.sync.dma_start(y_ap[tsl, osl], ysb[:])

    nc.compile()
    return nc


def _shard_inputs(inputs):
    x = np.asarray(inputs["x"])
    bias = np.asarray(inputs["attn_bias"])
    # layout-only host prep: bf16 casts and transposes (no arithmetic)
    xT_bf = np.ascontiguousarray(
        x.astype(BF16).transpose(0, 2, 1))          # [B, D, L]
    biasT = np.ascontiguousarray(
        bias[0].astype(BF16).transpose(0, 2, 1))    # [H, m, l]
    shared = {}
    for p in "qkvo":
        shared[f"WT{p}"] = np.ascontiguousarray(
            inputs[f"W{p}"].astype(BF16).T)          # [in, out]
        shared[f"AT{p}"] = np.ascontiguousarray(
            inputs[f"A{p}"].astype(BF16).T)          # [R, D]
        shared[f"B{p}"] = inputs[f"B{p}"].astype(BF16)
    shared["bq"] = np.asarray(inputs["bq"], np.float32)[:, None]
    shared["bk"] = np.asarray(inputs["bk"], np.float32)[:, None]
    shared["bv2"] = np.asarray(inputs["bv"], np.float32)[:, None]
    shared["bo"] = np.asarray(inputs["bo"], np.float32)[None, :]
    in_maps = []
    for c in range(N_CORES):
        b, qb = divmod(c, 4)
        qsl = slice(qb * QPC, (qb + 1) * QPC)
        m = dict(shared)
        m["xbT"] = xT_bf[b]
        m["xqT"] = np.ascontiguousarray(xT_bf[b][:, qsl])
        m["biasT"] = np.ascontiguousarray(biasT[:, :, qsl])
        in_maps.append(m)
    return in_maps


def _gather_outputs(results):
    y = np.empty((B, L, D), np.float32)
    for c in range(N_CORES):
        b, qb = divmod(c, 4)
        y[b, qb * QPC:(qb + 1) * QPC] = results[c]["y"]
    return y


def get_nc(**kw):
    key = ("nc", tuple(sorted(kw.items())))
    if key not in _CACHE:
        _CACHE[key] = _build_kernel(**kw)
    return _CACHE[key]


def build_runner(nc, n_cores=N_CORES):
    """Jitted SPMD executable for a prebuilt Bass module."""
    import jax
    from jax.sharding import Mesh, PartitionSpec
    from jax.experimental.shard_map import shard_map
    import concourse.mybir as mybir
    from concourse.bass2jax import (_bass_exec_p, install_neuronx_cc_hook,
                                    partition_id_tensor)

    install_neuronx_cc_hook()
    partition_name = (nc.partition_id_tensor.name
                      if nc.partition_id_tensor else None)
    in_names, out_names, out_avals, zero_outs = [], [], [], []
    for alloc in nc.m.functions[0].allocations:
        if not isinstance(alloc, mybir.MemoryLocationSet):
            continue
        name = alloc.memorylocations[0].name
        if alloc.kind == "ExternalInput":
            if name != partition_name:
                in_names.append(name)
        elif alloc.kind == "ExternalOutput":
            shape = tuple(alloc.tensor_shape)
            dtype = mybir.dt.np(alloc.dtype)
            out_names.append(name)
            out_avals.append(jax.core.ShapedArray(shape, dtype))
            zero_outs.append(np.zeros(shape, dtype))
    n_params = len(in_names)
    n_outs = len(out_avals)
    all_in_names = list(in_names) + list(out_names)
    if partition_name is not None:
        all_in_names.append(partition_name)

    def _body(*args):
        operands = list(args)
        if partition_name is not None:
            operands.append(partition_id_tensor())
        outs = _bass_exec_p.bind(
            *operands,
            out_avals=tuple(out_avals),
            in_names=tuple(all_in_names),
            out_names=tuple(out_names),
            lowering_input_output_aliases=(),
            sim_require_finite=True,
            sim_require_nnan=True,
            nc=nc,
        )
        return tuple(outs)

    devices = jax.devices()[:n_cores]
    mesh = Mesh(np.asarray(devices), ("core",))
    in_specs = (PartitionSpec("core"),) * (n_params + n_outs)
    out_specs = (PartitionSpec("core"),) * n_outs
    fn = jax.jit(shard_map(_body, mesh=mesh, in_specs=in_specs,
                           out_specs=out_specs, check_rep=False),
                 keep_unused=True)
    return fn, in_names, out_names, zero_outs


def _get_runner():
    if "runner" not in _CACHE:
        _CACHE["runner"] = build_runner(get_nc())
    return _CACHE["runner"]


def run_on_device(in_maps):
    import jax
    fn, in_names, out_names, zero_outs = _get_runner()
    concat_in = [np.concatenate([np.asarray(in_maps[c][nm])
                                 for c in range(N_CORES)], axis=0)
                 for nm in in_names]
    concat_zeros = [np.zeros((N_CORES * z.shape[0], *z.shape[1:]), z.dtype)
                    for z in zero_outs]
    out = fn(*concat_in, *concat_zeros)
    jax.block_until_ready(out)
    results = []
    for c in range(N_CORES):
        d = {}
        for i, nm in enumerate(out_names):
            arr = np.asarray(out[i])
            per = arr.shape[0] // N_CORES
            d[nm] = arr[c * per:(c + 1) * per]
        results.append(d)
    return results


def kernel(**inputs) -> np.ndarray:
    in_maps = _shard_inputs(inputs)
    results = run_on_device(in_maps)
    return _gather_outputs(results)
